# revision 60
# baseline (speedup 1.0000x reference)
"""Causal multi-head attention block (B=512, S=77, H=12, D=64, E=768) on 8 trn2 cores.

Data parallel over batch: 64 sequences per core, weights replicated.
Cost-model timeline: ~457 us per core (baseline 566 us); hw rel err 3.8e-4.

Per-core dataflow (chunks of 8 sequences = 616 tokens, 8 chunks):
  - x loaded token-major f32r, transposed on PE with an f32r identity
    (1.5 cyc/row vs 2.0 for f32) to feature-major x^T
  - q^T, k^T = W^T @ x^T feature-major in f32r (1 cyc/row at moving >= 256);
    scale+bias folded into the ACT PSUM->SBUF copies (q carries the 1/8
    attention scale); fp16 outputs with one zero pad column so the scores
    matmuls can stream SST=78 columns (initializes the psum pad column)
  - v computed token-major in 128-token tiles (stationary = x^T slices,
    moving = wv halves; 23k cyc/chunk vs 37k for per-seq tiles), copied
    PSUM->SBUF into per-tile staging with 65-column head stride: column 64
    of each head block is a ones column, so the attention-out matmul's 65th
    output row is the softmax denominator for free
  - staging is scattered into per-seq v tiles [77, 12*65] by gpsimd/SWDGE
    DMAs: engines cannot access partitions at unaligned starts, HWDGE
    dynamic DMAs drop descriptors when their scratch ring wraps under load,
    but the SWDGE path has ring flow control and is race-free
  - scoresT[t,s] per (seq, head); heads packed by parity into separate PSUM
    banks of one 2-bank tile (parity at column p*512; base-partition-0 and
    base-partition-64 matmuls race row-groups on a shared bank port)
  - unmasked exp straight from PSUM on ACT (scores are bounded, no
    max-subtraction), then one in-place all-fp16 multiplicative causal mask
    on DVE (2-byte fast path) -- the PE->DVE->ACT->PE chain of the additive
    mask becomes PE->ACT->DVE->PE with ~half the latency
  - out matmuls consume the UNNORMALIZED masked exp with the augmented v as
    stationary -> [65, 78] per head: rows 0-63 = attn-out^T, row 64 = denom
  - reciprocal on DVE (fp16), broadcast across the 64 d-partitions on the
    otherwise-idle Pool engine (partition_broadcast), normalization
    multiplied in at the DVE PSUM->SBUF move of the attention output -> the
    PE never touches denominators or broadcasts
  - v bias folded through attention (softmax rows sum to 1):
    y = (attn @ v) @ wo + (bv @ wo + bo), bv@wo computed on-device
  - final projection in fp16 (ao fp16 stationary, wo fp16 moving, loaded
    via casting DMAs so no ACT convert blocks on the late wo load); bo
    added by DVE; y stores issued right after their y-add so their
    data-ready wait never starves a DMA queue

Scheduling: per-chunk attention batches are the spine (scores of batch b+1
emitted before out of batch b); projection work of chunk c+1 fills between
them, x-transpose tiles interleaved with v tiles to keep the transpose psum
ping-pong fed; final tiles of chunk c-1 are inserted as soon as the
attention batches they read have been emitted, with the last tiles carried
to the next step's head as dependency-free tail fill.
"""

import sys

sys.path.insert(0, "/opt/trn_rl_repo")

import numpy as np
from contextlib import ExitStack

import concourse.bass as bass
import concourse.tile as tile
from concourse import bacc, mybir
from concourse.bass_utils import run_bass_kernel_spmd
from concourse.masks import make_identity

B, S, H, D = 512, 77, 12, 64
E = H * D  # 768
NCORES = 8
B_LOC = B // NCORES  # 64
NTOK = B_LOC * S  # 4928
CHUNK_B = 8
CHUNK_TOK = CHUNK_B * S  # 616
NCHUNK = B_LOC // CHUNK_B  # 8
KC = E // 128  # 6 k-chunks of 128
F32 = mybir.dt.float32
F32R = mybir.dt.float32r
FP16 = mybir.dt.float16
ATTN_DT = FP16
SCALE = 0.125
MASK_VAL = -1e9
SST = 78  # head block stride in scores layout (8B-aligned psum columns)
HHALF = 6 * SST  # 468, six heads per psum bank
VST = 65  # head stride in v_aug (64 v columns + ones column)
VW = H * VST  # 780

# token tiles within a chunk
TT = [(0, 128), (128, 128), (256, 128), (384, 128), (512, CHUNK_TOK - 512)]

ALU = mybir.AluOpType
AF = mybir.ActivationFunctionType


def bcast_ap(handle_ap, parts, n):
    """DRAM [n] vector viewed as [parts, n] with partition step 0."""
    return bass.AP(
        tensor=handle_ap.tensor,
        offset=handle_ap.offset,
        ap=[[0, parts]] + list(handle_ap.ap),
    )


def _seq_pieces(toff, tw):
    """Split chunk-token rows [toff, toff+tw) at sequence boundaries.
    Returns (seq_idx, row0_in_tile, row0_in_seq, nrows) pieces."""
    out = []
    r = toff
    while r < toff + tw:
        sq = r // S
        rs = r % S
        n = min(S - rs, toff + tw - r)
        out.append((sq, r - toff, rs, n))
        r += n
    return out


def build_nc():
    nc = bacc.Bacc("TRN2", target_bir_lowering=False)
    x = nc.dram_tensor("x", [NTOK, E], F32R, kind="ExternalInput").ap()
    wq = nc.dram_tensor("wq", [E, E], F32R, kind="ExternalInput").ap()
    wk = nc.dram_tensor("wk", [E, E], F32R, kind="ExternalInput").ap()
    wv = nc.dram_tensor("wv", [E, E], F32R, kind="ExternalInput").ap()
    wo = nc.dram_tensor("wo", [E, E], F32R, kind="ExternalInput").ap()
    bq = nc.dram_tensor("bq", [E], F32, kind="ExternalInput").ap()
    bk = nc.dram_tensor("bk", [E], F32, kind="ExternalInput").ap()
    bv = nc.dram_tensor("bv", [E], F32, kind="ExternalInput").ap()
    bo = nc.dram_tensor("bo", [E], F32, kind="ExternalInput").ap()
    out = nc.dram_tensor("out", [NTOK, E], F32, kind="ExternalOutput").ap()

    with tile.TileContext(nc) as tc, ExitStack() as ctx:
        singles = ctx.enter_context(tc.tile_pool(name="singles", bufs=1))
        xtokp = ctx.enter_context(tc.tile_pool(name="xtok", bufs=4))
        xtp = ctx.enter_context(tc.tile_pool(name="xt", bufs=2))
        qkp = ctx.enter_context(tc.tile_pool(name="qk", bufs=4))
        vp = ctx.enter_context(tc.tile_pool(name="v", bufs=16))
        vtp = ctx.enter_context(tc.tile_pool(name="vtok", bufs=3))
        aop = ctx.enter_context(tc.tile_pool(name="ao", bufs=2))
        scp = ctx.enter_context(tc.tile_pool(name="sc", bufs=2))
        rcp = ctx.enter_context(tc.tile_pool(name="rc", bufs=2))
        yp = ctx.enter_context(tc.tile_pool(name="y", bufs=2))
        ps1 = ctx.enter_context(tc.tile_pool(name="ps1", bufs=4, space="PSUM"))
        pss = ctx.enter_context(tc.tile_pool(name="pss", bufs=1, space="PSUM"))
        psa = ctx.enter_context(tc.tile_pool(name="psa", bufs=2, space="PSUM"))

        # ---- identity first: the first transposes must not wait on the
        # weight/bias DMA preamble ----
        ident = singles.tile([128, 128], F32, tag="ident", name="ident")
        make_identity(nc, ident[:])
        identr = singles.tile([128, 128], F32R, tag="identr", name="identr")
        nc.vector.tensor_copy(identr[:], ident[:])

        # ---- prefetch chunk 0 x tiles before the (large) weight DMAs ----
        xtok0 = []
        for toff, tw in TT:
            xtok = xtokp.tile([128, E], F32R, tag="xtok", name="xtok")
            nc.sync.dma_start(xtok[0:tw, :], x[toff : toff + tw, :])
            xtok0.append(xtok)

        # ---- weights ----
        w_sb = {}
        for name, w in (("wq", wq), ("wk", wk), ("wv", wv)):
            tiles = []
            for kc in range(KC):
                t = singles.tile([128, E], F32R, tag=f"{name}{kc}", name=f"{name}{kc}")
                nc.sync.dma_start(t[:], w[kc * 128 : (kc + 1) * 128, :])
                tiles.append(t)
            w_sb[name] = tiles
        # wo straight to fp16 via casting DMAs on the SWDGE path (keeps the
        # ACT queue free of converts that would head-of-line block on the
        # late wo DMA)
        wo16 = []
        for kc in range(KC):
            t16 = singles.tile([128, E], FP16, tag=f"wo16_{kc}", name=f"wo16_{kc}")
            nc.gpsimd.dma_start(t16[:], wo[kc * 128 : (kc + 1) * 128, :])
            wo16.append(t16)

        bq_col = singles.tile([128, KC], F32, tag="bqc", name="bqc")
        bk_col = singles.tile([128, KC], F32, tag="bkc", name="bkc")
        nc.gpsimd.dma_start(bq_col[:], bq.rearrange("(f p) -> p f", p=128))
        nc.gpsimd.dma_start(bk_col[:], bk.rearrange("(f p) -> p f", p=128))
        # fold the attention scale into the q bias: q = (x@wq)*s + bq*s
        nc.vector.tensor_scalar_mul(bq_col[:], bq_col[:], SCALE)

        bv_col = singles.tile([128, KC], FP16, tag="bvc", name="bvc")
        nc.gpsimd.dma_start(bv_col[:], bv.rearrange("(f p) -> p f", p=128))
        bo_bc = singles.tile([128, E], F32, tag="bob", name="bob")
        nc.gpsimd.dma_start(bo_bc[:], bcast_ap(bo, 128, E))

        # multiplicative causal mask in [t, (h s)] layout, fp16:
        # 1 where t <= s else 0 (applied AFTER the exp; scores are bounded so
        # the unmasked exp cannot overflow)
        mask16 = singles.tile([S, H, SST], FP16, tag="mask16", name="mask16")
        nc.gpsimd.memset(mask16[:], 1.0)
        nc.gpsimd.affine_select(
            out=mask16[:],
            in_=mask16[:],
            compare_op=ALU.is_ge,
            fill=0.0,
            base=0,
            pattern=[[0, H], [1, SST]],
            channel_multiplier=-1,
        )
        mask16_flat = mask16[:].rearrange("t h s -> t (h s)")

        # softmax rows sum to 1, so the v bias passes through attention:
        # y = (attn @ v) @ wo + (bv @ wo + bo).  Fold bv@wo into the bias.
        bvwo_row = singles.tile([1, E], F32, tag="bvwo", name="bvwo")

        def emit_bias_fold():
            for half in range(2):
                pbw = ps1.tile([1, 384], F32, tag="p", name="p")
                for kc in range(KC):
                    nc.tensor.matmul(
                        pbw[:], bv_col[:, kc : kc + 1],
                        wo16[kc][:, half * 384 : (half + 1) * 384],
                        start=(kc == 0), stop=(kc == KC - 1),
                    )
                nc.vector.tensor_copy(
                    bvwo_row[:, half * 384 : (half + 1) * 384], pbw[:]
                )
            bvwo_bc = yp.tile([128, E], F32, tag="y", name="bvwo_bc")
            nc.gpsimd.partition_broadcast(bvwo_bc[:], bvwo_row[:])
            nc.vector.tensor_add(bo_bc[:], bo_bc[:], bvwo_bc[:])

        # ---- per-chunk pieces ----
        def emit_load_qkv(c):
            ctok = c * CHUNK_TOK
            st = {}
            pieces = []

            xt_t = xtp.tile([128, KC, CHUNK_TOK], F32R, tag="xt", name="xt")
            st["xt"] = [xt_t[:, kc, :] for kc in range(KC)]
            st["vaug"] = [
                vp.tile([S, VW], ATTN_DT, tag="v", name="v") for _ in range(CHUNK_B)
            ]
            st["vtok"] = {}

            def p_transpose(ti):
                def run():
                    toff, tw = TT[ti]
                    if c == 0:
                        xtok = xtok0[ti]
                    else:
                        xtok = xtokp.tile([128, E], F32R, tag="xtok", name="xtok")
                        nc.sync.dma_start(
                            xtok[0:tw, :], x[ctok + toff : ctok + toff + tw, :]
                        )
                    for g in range(2):
                        tp = ps1.tile([128, 384], F32R, tag="p", name="tp")
                        for j in range(3):
                            kc = 3 * g + j
                            nc.tensor.transpose(
                                tp[:, j * 128 : j * 128 + tw],
                                xtok[0:tw, kc * 128 : (kc + 1) * 128],
                                identr[0:tw, 0:tw],
                            )
                        # drain each transpose psum with DVE and ACT in
                        # parallel halves so the 2-group ping-pong frees fast
                        src = tp[:].rearrange("p (j c) -> p j c", c=128)[:, :, 0:tw]
                        dst = xt_t[:, 3 * g : 3 * g + 3, toff : toff + tw]
                        nc.vector.tensor_copy(dst[:, 0:2, :], src[:, 0:2, :])
                        nc.scalar.copy(dst[:, 2:3, :], src[:, 2:3, :])
                return run

            def p_proj(wname, dkey, bias, scale, ec, th):
                def run():
                    if dkey not in st:
                        st[dkey] = qkp.tile(
                            [128, KC, CHUNK_TOK + 1], ATTN_DT, tag="qk", name=dkey
                        )
                        # the scores matmuls stream 78 moving columns so the
                        # psum pad column is initialized; the last batch
                        # needs this extra zero column
                        nc.vector.memset(
                            st[dkey][:, :, CHUNK_TOK : CHUNK_TOK + 1], 0.0
                        )
                    dst = st[dkey]
                    t0 = th * 308
                    ps = ps1.tile([128, 308], F32, tag="p", name="p")
                    for kc in range(KC):
                        nc.tensor.matmul(
                            ps[:],
                            w_sb[wname][kc][:, ec * 128 : (ec + 1) * 128],
                            st["xt"][kc][:, t0 : t0 + 308],
                            start=(kc == 0),
                            stop=(kc == KC - 1),
                        )
                    nc.scalar.activation(
                        dst[:, ec, t0 : t0 + 308], ps[:], AF.Identity,
                        bias=bias[:, ec : ec + 1], scale=scale,
                    )
                return run

            def p_v(ti, half):
                def run():
                    toff, tw = TT[ti]
                    if ti not in st["vtok"]:
                        vt = vtp.tile([128, VW], ATTN_DT, tag="vtok", name="vtok")
                        st["vtok"][ti] = vt
                        # ones columns written here; the per-seq scatter DMAs
                        # copy full 780-wide rows so they carry the ones along
                        nc.gpsimd.memset(
                            vt[:].rearrange("t (h v) -> t h v", v=VST)[:, :, 64:65],
                            1.0,
                        )
                    vt = st["vtok"][ti]
                    pv = ps1.tile([128, 384], F32, tag="p", name="p")
                    for kc in range(KC):
                        nc.tensor.matmul(
                            pv[0:tw, :],
                            st["xt"][kc][:, toff : toff + tw],
                            w_sb["wv"][kc][:, half * 384 : (half + 1) * 384],
                            start=(kc == 0),
                            stop=(kc == KC - 1),
                        )
                    # partition-aligned PSUM->SBUF move (engines cannot start
                    # at an unaligned partition, so no per-seq split here)
                    nc.scalar.copy(
                        vt[0:tw].rearrange("t (h v) -> t h v", v=VST)[
                            :, half * 6 : half * 6 + 6, 0:64
                        ],
                        pv[0:tw].rearrange("t (h v) -> t h v", v=64),
                    )
                return run

            def p_vscatter(ti):
                def run():
                    toff, tw = TT[ti]
                    vt = st["vtok"].pop(ti)
                    # DMAs have no partition-alignment rules. SWDGE (gpsimd)
                    # descriptor generation has ring flow control (blocks when
                    # full), unlike the HWDGE dynamic path whose ring wrap
                    # clobbers in-flight descriptors.
                    for sq, r0, rs, n in _seq_pieces(toff, tw):
                        nc.gpsimd.dma_start(
                            st["vaug"][sq][rs : rs + n, :], vt[r0 : r0 + n, :]
                        )
                return run

            if c == 0:
                # startup: weights arrive in wq, wk, wv order over ~25us of
                # serialized DMA -- consume them in that order
                pieces = [p_transpose(ti) for ti in range(len(TT))]
                for th in range(2):
                    for ec in range(KC):
                        pieces.append(p_proj("wq", "q_sb", bq_col, SCALE, ec, th))
                for th in range(2):
                    for ec in range(KC):
                        pieces.append(p_proj("wk", "k_sb", bk_col, 1.0, ec, th))
                for ti in range(len(TT)):
                    pieces.append(p_v(ti, 0))
                    pieces.append(p_v(ti, 1))
                    pieces.append(p_vscatter(ti))
                return st, pieces, []
            # steady state: each v tile directly behind its own x-transpose
            # tile so the transpose psum ping-pong always has matmul work
            # between drains; q/k th=0 only needs transpose tiles 0-2
            pieces = [
                p_transpose(0), p_transpose(1),
                p_v(0, 0), p_transpose(2), p_v(0, 1), p_vscatter(0),
                p_v(1, 0), p_transpose(3), p_v(1, 1), p_vscatter(1),
                p_v(2, 0), p_transpose(4), p_v(2, 1), p_vscatter(2),
            ]
            pieces.extend([p_v(3, 0), p_v(3, 1), p_vscatter(3)])
            pieces.extend([p_v(4, 0), p_v(4, 1), p_vscatter(4)])
            for ec in range(KC):
                pieces.append(p_proj("wq", "q_sb", bq_col, SCALE, ec, 0))
                pieces.append(p_proj("wk", "k_sb", bk_col, 1.0, ec, 0))
            for ec in range(KC):
                pieces.append(p_proj("wq", "q_sb", bq_col, SCALE, ec, 1))
                pieces.append(p_proj("wk", "k_sb", bk_col, 1.0, ec, 1))
            return st, pieces, []

        def emit_attn(c, st):
            ast = {}
            ao_t = aop.tile([128, KC, CHUNK_TOK], ATTN_DT, tag="ao", name="ao")
            ast["ao"] = ao_t
            bstate = {}

            def p_scores(bb):
                def run():
                    q_sb, k_sb = st["q_sb"], st["k_sb"]
                    boff = bb * S
                    # one 2-bank psum tile; parity p at column p*512 so every
                    # per-head matmul output stays inside a single bank
                    sps = pss.tile([S, 1024], F32, tag="s", name="s")
                    for kc in range(KC):
                        for par in range(2):
                            nc.tensor.matmul(
                                sps[:, par * 512 + kc * SST : par * 512 + kc * SST + SST],
                                k_sb[par * 64 : par * 64 + 64, kc, boff : boff + S],
                                q_sb[par * 64 : par * 64 + 64, kc, boff : boff + SST],
                                start=True,
                                stop=True,
                            )
                    # unmasked exp straight from PSUM (one wide ACT op), then
                    # one in-place all-fp16 mask multiply on DVE (fast path)
                    sc = scp.tile([S, 2 * HHALF], ATTN_DT, tag="sc", name="sc")
                    nc.scalar.activation(
                        sc[:].rearrange("t (p c) -> t p c", p=2),
                        sps[:].rearrange("t (p c) -> t p c", p=2)[:, :, 0:HHALF],
                        AF.Exp,
                    )
                    nc.vector.tensor_mul(sc[:], sc[:], mask16_flat[:])
                    bstate[bb] = sc
                return run

            def p_out(bb):
                def run():
                    sc = bstate.pop(bb)
                    vaug = st["vaug"][bb]
                    boff = bb * S
                    aps = [
                        psa.tile([VST, HHALF], F32, tag="a", name="a")
                        for _ in range(2)
                    ]
                    for kc in range(KC):
                        for par in range(2):
                            h = 2 * kc + par
                            nc.tensor.matmul(
                                aps[par][:, kc * SST : kc * SST + SST],
                                vaug[0:S, h * VST : (h + 1) * VST],
                                sc[:, par * HHALF + kc * SST : par * HHALF + kc * SST + SST],
                                start=True,
                                stop=True,
                            )
                    recip = rcp.tile([1, 2 * HHALF], ATTN_DT, tag="recip", name="recip")
                    recbc = rcp.tile([64, 2 * HHALF], ATTN_DT, tag="recbc", name="recbc")
                    with nc.allow_low_precision(reason="softmax denom rounding"):
                        for par in range(2):
                            nc.vector.reciprocal(
                                recip[:, par * HHALF : (par + 1) * HHALF],
                                aps[par][64:65, :],
                            )
                    for par in range(2):
                        nc.gpsimd.partition_broadcast(
                            recbc[:, par * HHALF : (par + 1) * HHALF],
                            recip[:, par * HHALF : (par + 1) * HHALF],
                        )
                    for par in range(2):
                        nc.vector.tensor_mul(
                            ao_t[par * 64 : par * 64 + 64, :, boff : boff + S],
                            aps[par][0:64, :].rearrange(
                                "p (j s) -> p j s", s=SST
                            )[:, :, 0:S],
                            recbc[:, par * HHALF : (par + 1) * HHALF].rearrange(
                                "p (j s) -> p j s", s=SST
                            )[:, :, 0:S],
                        )
                return run

            s_pieces = [p_scores(bb) for bb in range(CHUNK_B)]
            o_pieces = [p_out(bb) for bb in range(CHUNK_B)]
            return ast, s_pieces, o_pieces

        def emit_final(c, ast):
            ctok = c * CHUNK_TOK
            ao = ast["ao"]

            def p_tt(ti):
                def run():
                    toff, tw = TT[ti]
                    yt = yp.tile([128, E], F32, tag="y", name="y")
                    for half in range(2):
                        yps = ps1.tile([128, 384], F32, tag="p", name="p")
                        for kc in range(KC):
                            nc.tensor.matmul(
                                yps[0:tw, :],
                                ao[:, kc, toff : toff + tw],
                                wo16[kc][:, half * 384 : (half + 1) * 384],
                                start=(kc == 0),
                                stop=(kc == KC - 1),
                            )
                        nc.vector.tensor_add(
                            yt[0:tw, half * 384 : (half + 1) * 384],
                            yps[0:tw, :],
                            bo_bc[0:tw, half * 384 : (half + 1) * 384],
                        )
                    # store on SP: emitted right after the y-add, so its
                    # data-ready wait is short and cannot starve the queue
                    # (keeping it off Pool protects the bcast/memset stream)
                    nc.gpsimd.dma_start(
                        out[ctok + toff : ctok + toff + tw, :], yt[0:tw, :]
                    )
                return run

            return [p_tt(ti) for ti in range(len(TT))]

        # ---- interleave: attention batches of chunk c-1 are the spine;
        # projections of chunk c fill between; final tiles of chunk c-1 are
        # inserted as soon as the attention batches they read have been
        # emitted (one batch of slack for the normalize chain); the last two
        # final tiles carry over to the next step's head as tail fill ----
        qkv_st = {}
        held = []
        for c in range(NCHUNK + 1):
            fill = list(held)
            held = []
            spine = []
            after_o = {}
            if c < NCHUNK:
                qkv_st[c], p, ptail = emit_load_qkv(c)
                fill.extend(p)
                fill.extend(ptail)
            if 1 <= c <= NCHUNK:
                ast, s_p, o_p = emit_attn(c - 1, qkv_st.pop(c - 1))
                f = emit_final(c - 1, ast)
                # final tile ti reads ao of seqs up to (toff+tw-1)//S; insert
                # one O later for the DVE normalize to land; hold the last two
                ins_at = []
                for ti, (toff, tw) in enumerate(TT):
                    ins_at.append(min((toff + tw - 1) // S + 1, CHUNK_B - 1))
                for ti in range(2):
                    after_o.setdefault(ins_at[ti], []).append(f[ti])
                held.extend(f[2:])
                # spine: scores(bb+1) emitted before out(bb)
                spine = [("S", 0, s_p[0])]
                for bb in range(1, CHUNK_B):
                    spine.append(("S", bb, s_p[bb]))
                    spine.append(("O", bb - 1, o_p[bb - 1]))
                spine.append(("O", CHUNK_B - 1, o_p[CHUNK_B - 1]))

            if not spine:
                for p_ in fill:
                    p_()
            else:
                # x DMA + first transpose early
                nhead = min(1, len(fill))
                for p_ in fill[:nhead]:
                    p_()
                rest = fill[nhead:]
                k = 0
                for i, (kind, bb, sp) in enumerate(spine):
                    sp()
                    if kind == "O":
                        for fp_ in after_o.get(bb, ()):
                            fp_()
                    share = ((i + 1) * len(rest)) // len(spine)
                    while k < share:
                        rest[k]()
                        k += 1
                while k < len(rest):
                    rest[k]()
                    k += 1
            if c == 0:
                emit_bias_fold()
        for p_ in held:
            p_()

    nc.finalize()
    return nc


_NC_CACHE = {}


def get_nc():
    if "nc" not in _NC_CACHE:
        _NC_CACHE["nc"] = build_nc()
    return _NC_CACHE["nc"]


def kernel(**inputs):
    x = np.asarray(inputs["x"], dtype=np.float32)  # [512, 77, 768]
    nc = get_nc()
    shared = {
        k: np.asarray(inputs[k], dtype=np.float32)
        for k in ("wq", "bq", "wk", "bk", "wv", "bv", "wo", "bo")
    }
    in_maps = []
    for c in range(NCORES):
        m = dict(shared)
        m["x"] = np.ascontiguousarray(
            x[c * B_LOC : (c + 1) * B_LOC].reshape(NTOK, E)
        )
        in_maps.append(m)
    res = run_bass_kernel_spmd(nc, in_maps, core_ids=list(range(NCORES)))
    out = np.concatenate(
        [r_["out"].reshape(B_LOC, S, E) for r_ in res.results], axis=0
    )
    return out


# revision 67
# speedup vs baseline: 1.0218x; 1.0218x over previous
"""Causal multi-head attention block (B=512, S=77, H=12, D=64, E=768) on 8 trn2 cores.

Data parallel over batch: 64 sequences per core, weights replicated.
Cost-model timeline: ~457 us per core (baseline 566 us); hw rel err 3.8e-4.

Per-core dataflow (chunks of 8 sequences = 616 tokens, 8 chunks):
  - x loaded token-major f32r, transposed on PE with an f32r identity
    (1.5 cyc/row vs 2.0 for f32) to feature-major x^T
  - q^T, k^T = W^T @ x^T feature-major in f32r (1 cyc/row at moving >= 256);
    scale+bias folded into the ACT PSUM->SBUF copies (q carries the 1/8
    attention scale); fp16 outputs with one zero pad column so the scores
    matmuls can stream SST=78 columns (initializes the psum pad column)
  - v computed token-major in 128-token tiles (stationary = x^T slices,
    moving = wv halves; 23k cyc/chunk vs 37k for per-seq tiles), copied
    PSUM->SBUF into per-tile staging with 65-column head stride: column 64
    of each head block is a ones column, so the attention-out matmul's 65th
    output row is the softmax denominator for free
  - staging is scattered into per-seq v tiles [77, 12*65] by gpsimd/SWDGE
    DMAs: engines cannot access partitions at unaligned starts, HWDGE
    dynamic DMAs drop descriptors when their scratch ring wraps under load,
    but the SWDGE path has ring flow control and is race-free
  - scoresT[t,s] per (seq, head); heads packed by parity into separate PSUM
    banks of one 2-bank tile (parity at column p*512; base-partition-0 and
    base-partition-64 matmuls race row-groups on a shared bank port)
  - unmasked exp straight from PSUM on ACT (scores are bounded, no
    max-subtraction), then one in-place all-fp16 multiplicative causal mask
    on DVE (2-byte fast path) -- the PE->DVE->ACT->PE chain of the additive
    mask becomes PE->ACT->DVE->PE with ~half the latency
  - out matmuls consume the UNNORMALIZED masked exp with the augmented v as
    stationary -> [65, 78] per head: rows 0-63 = attn-out^T, row 64 = denom
  - reciprocal on DVE (fp16), broadcast across the 64 d-partitions on the
    otherwise-idle Pool engine (partition_broadcast), normalization
    multiplied in at the DVE PSUM->SBUF move of the attention output -> the
    PE never touches denominators or broadcasts
  - v bias folded through attention (softmax rows sum to 1):
    y = (attn @ v) @ wo + (bv @ wo + bo), bv@wo computed on-device
  - final projection in fp16 (ao fp16 stationary, wo fp16 moving, loaded
    via casting DMAs so no ACT convert blocks on the late wo load); bo
    added by DVE; y stores issued right after their y-add so their
    data-ready wait never starves a DMA queue

Scheduling: per-chunk attention batches are the spine (scores of batch b+1
emitted before out of batch b); projection work of chunk c+1 fills between
them, x-transpose tiles interleaved with v tiles to keep the transpose psum
ping-pong fed; final tiles of chunk c-1 are inserted as soon as the
attention batches they read have been emitted, with the last tiles carried
to the next step's head as dependency-free tail fill.
"""

import sys

sys.path.insert(0, "/opt/trn_rl_repo")

import numpy as np
from contextlib import ExitStack

import concourse.bass as bass
import concourse.tile as tile
from concourse import bacc, mybir
from concourse.bass_utils import run_bass_kernel_spmd
from concourse.masks import make_identity

B, S, H, D = 512, 77, 12, 64
E = H * D  # 768
NCORES = 8
B_LOC = B // NCORES  # 64
NTOK = B_LOC * S  # 4928
CHUNK_B = 8
CHUNK_TOK = CHUNK_B * S  # 616
NCHUNK = B_LOC // CHUNK_B  # 8
KC = E // 128  # 6 k-chunks of 128
F32 = mybir.dt.float32
F32R = mybir.dt.float32r
FP16 = mybir.dt.float16
ATTN_DT = FP16
SCALE = 0.125
MASK_VAL = -1e9
SST = 78  # head block stride in scores layout (8B-aligned psum columns)
HHALF = 6 * SST  # 468, six heads per psum bank
VST = 65  # head stride in v_aug (64 v columns + ones column)
VW = H * VST  # 780

# token tiles within a chunk
TT = [(0, 128), (128, 128), (256, 128), (384, 128), (512, CHUNK_TOK - 512)]

ALU = mybir.AluOpType
AF = mybir.ActivationFunctionType


def bcast_ap(handle_ap, parts, n):
    """DRAM [n] vector viewed as [parts, n] with partition step 0."""
    return bass.AP(
        tensor=handle_ap.tensor,
        offset=handle_ap.offset,
        ap=[[0, parts]] + list(handle_ap.ap),
    )


def _seq_pieces(toff, tw):
    """Split chunk-token rows [toff, toff+tw) at sequence boundaries.
    Returns (seq_idx, row0_in_tile, row0_in_seq, nrows) pieces."""
    out = []
    r = toff
    while r < toff + tw:
        sq = r // S
        rs = r % S
        n = min(S - rs, toff + tw - r)
        out.append((sq, r - toff, rs, n))
        r += n
    return out


def build_nc():
    nc = bacc.Bacc("TRN2", target_bir_lowering=False)
    x = nc.dram_tensor("x", [NTOK, E], F32R, kind="ExternalInput").ap()
    wq = nc.dram_tensor("wq", [E, E], F32R, kind="ExternalInput").ap()
    wk = nc.dram_tensor("wk", [E, E], F32R, kind="ExternalInput").ap()
    wv = nc.dram_tensor("wv", [E, E], F32R, kind="ExternalInput").ap()
    wo = nc.dram_tensor("wo", [E, E], F32R, kind="ExternalInput").ap()
    bq = nc.dram_tensor("bq", [E], F32, kind="ExternalInput").ap()
    bk = nc.dram_tensor("bk", [E], F32, kind="ExternalInput").ap()
    bv = nc.dram_tensor("bv", [E], F32, kind="ExternalInput").ap()
    bo = nc.dram_tensor("bo", [E], F32, kind="ExternalInput").ap()
    out = nc.dram_tensor("out", [NTOK, E], F32, kind="ExternalOutput").ap()

    with tile.TileContext(nc) as tc, ExitStack() as ctx:
        singles = ctx.enter_context(tc.tile_pool(name="singles", bufs=1))
        xtokp = ctx.enter_context(tc.tile_pool(name="xtok", bufs=4))
        xtp = ctx.enter_context(tc.tile_pool(name="xt", bufs=2))
        qkp = ctx.enter_context(tc.tile_pool(name="qk", bufs=4))
        vp = ctx.enter_context(tc.tile_pool(name="v", bufs=16))
        vtp = ctx.enter_context(tc.tile_pool(name="vtok", bufs=3))
        aop = ctx.enter_context(tc.tile_pool(name="ao", bufs=2))
        scp = ctx.enter_context(tc.tile_pool(name="sc", bufs=2))
        rcp = ctx.enter_context(tc.tile_pool(name="rc", bufs=2))
        yp = ctx.enter_context(tc.tile_pool(name="y", bufs=2))
        ps1 = ctx.enter_context(tc.tile_pool(name="ps1", bufs=4, space="PSUM"))
        pss = ctx.enter_context(tc.tile_pool(name="pss", bufs=1, space="PSUM"))
        psa = ctx.enter_context(tc.tile_pool(name="psa", bufs=2, space="PSUM"))

        # ---- identity first: the first transposes must not wait on the
        # weight/bias DMA preamble ----
        ident = singles.tile([128, 128], F32, tag="ident", name="ident")
        make_identity(nc, ident[:])
        identr = singles.tile([128, 128], F32R, tag="identr", name="identr")
        nc.vector.tensor_copy(identr[:], ident[:])

        # ---- prefetch chunk 0 x tiles before the (large) weight DMAs ----
        xtok0 = []
        for toff, tw in TT:
            xtok = xtokp.tile([128, E], F32R, tag="xtok", name="xtok")
            nc.sync.dma_start(xtok[0:tw, :], x[toff : toff + tw, :])
            xtok0.append(xtok)

        # ---- weights ----
        w_sb = {}
        for name, w in (("wq", wq), ("wk", wk), ("wv", wv)):
            tiles = []
            for kc in range(KC):
                t = singles.tile([128, E], F32R, tag=f"{name}{kc}", name=f"{name}{kc}")
                nc.sync.dma_start(t[:], w[kc * 128 : (kc + 1) * 128, :])
                tiles.append(t)
            w_sb[name] = tiles
        # wo straight to fp16 via casting DMAs on the SWDGE path (keeps the
        # ACT queue free of converts that would head-of-line block on the
        # late wo DMA)
        wo16 = []
        for kc in range(KC):
            t16 = singles.tile([128, E], FP16, tag=f"wo16_{kc}", name=f"wo16_{kc}")
            nc.gpsimd.dma_start(t16[:], wo[kc * 128 : (kc + 1) * 128, :])
            wo16.append(t16)

        bq_col = singles.tile([128, KC], F32, tag="bqc", name="bqc")
        bk_col = singles.tile([128, KC], F32, tag="bkc", name="bkc")
        nc.gpsimd.dma_start(bq_col[:], bq.rearrange("(f p) -> p f", p=128))
        nc.gpsimd.dma_start(bk_col[:], bk.rearrange("(f p) -> p f", p=128))
        # fold the attention scale into the q bias: q = (x@wq)*s + bq*s
        nc.vector.tensor_scalar_mul(bq_col[:], bq_col[:], SCALE)

        bv_col = singles.tile([128, KC], FP16, tag="bvc", name="bvc")
        nc.gpsimd.dma_start(bv_col[:], bv.rearrange("(f p) -> p f", p=128))
        bo_bc = singles.tile([128, E], F32, tag="bob", name="bob")
        nc.gpsimd.dma_start(bo_bc[:], bcast_ap(bo, 128, E))

        # multiplicative causal mask in [t, (h s)] layout, fp16:
        # 1 where t <= s else 0 (applied AFTER the exp; scores are bounded so
        # the unmasked exp cannot overflow)
        mask16 = singles.tile([S, H, SST], FP16, tag="mask16", name="mask16")
        nc.gpsimd.memset(mask16[:], 1.0)
        nc.gpsimd.affine_select(
            out=mask16[:],
            in_=mask16[:],
            compare_op=ALU.is_ge,
            fill=0.0,
            base=0,
            pattern=[[0, H], [1, SST]],
            channel_multiplier=-1,
        )
        mask16_flat = mask16[:].rearrange("t h s -> t (h s)")

        # softmax rows sum to 1, so the v bias passes through attention:
        # y = (attn @ v) @ wo + (bv @ wo + bo).  Fold bv@wo into the bias.
        bvwo_row = singles.tile([1, E], F32, tag="bvwo", name="bvwo")

        def emit_bias_fold():
            for half in range(2):
                pbw = ps1.tile([1, 384], F32, tag="p", name="p")
                for kc in range(KC):
                    nc.tensor.matmul(
                        pbw[:], bv_col[:, kc : kc + 1],
                        wo16[kc][:, half * 384 : (half + 1) * 384],
                        start=(kc == 0), stop=(kc == KC - 1),
                    )
                nc.vector.tensor_copy(
                    bvwo_row[:, half * 384 : (half + 1) * 384], pbw[:]
                )
            bvwo_bc = yp.tile([128, E], F32, tag="y", name="bvwo_bc")
            nc.gpsimd.partition_broadcast(bvwo_bc[:], bvwo_row[:])
            nc.vector.tensor_add(bo_bc[:], bo_bc[:], bvwo_bc[:])

        # ---- per-chunk pieces ----
        def emit_load_qkv(c):
            ctok = c * CHUNK_TOK
            st = {}
            pieces = []

            xt_t = xtp.tile([128, KC, CHUNK_TOK], F32R, tag="xt", name="xt")
            st["xt"] = [xt_t[:, kc, :] for kc in range(KC)]
            st["vaug"] = [
                vp.tile([S, VW], ATTN_DT, tag="v", name="v") for _ in range(CHUNK_B)
            ]
            st["vtok"] = {}

            def p_transpose(ti):
                def run():
                    toff, tw = TT[ti]
                    if c == 0:
                        xtok = xtok0[ti]
                    else:
                        xtok = xtokp.tile([128, E], F32R, tag="xtok", name="xtok")
                        nc.sync.dma_start(
                            xtok[0:tw, :], x[ctok + toff : ctok + toff + tw, :]
                        )
                    for g in range(3):
                        tp = ps1.tile([128, 256], F32R, tag="p", name="tp")
                        for j in range(2):
                            kc = 2 * g + j
                            nc.tensor.transpose(
                                tp[:, j * 128 : j * 128 + tw],
                                xtok[0:tw, kc * 128 : (kc + 1) * 128],
                                identr[0:tw, 0:tw],
                            )
                        # small psum groups + alternating drain engines keep
                        # the transpose ping-pong fed
                        src = tp[:].rearrange("p (j c) -> p j c", c=128)[:, :, 0:tw]
                        dst = xt_t[:, 2 * g : 2 * g + 2, toff : toff + tw]
                        if g % 2 == 0:
                            nc.vector.tensor_copy(dst, src)
                        else:
                            nc.scalar.copy(dst, src)
                return run

            def p_proj(wname, dkey, bias, scale, ec, th):
                def run():
                    if dkey not in st:
                        st[dkey] = qkp.tile(
                            [128, KC, CHUNK_TOK + 1], ATTN_DT, tag="qk", name=dkey
                        )
                        # the scores matmuls stream 78 moving columns so the
                        # psum pad column is initialized; the last batch
                        # needs this extra zero column
                        nc.vector.memset(
                            st[dkey][:, :, CHUNK_TOK : CHUNK_TOK + 1], 0.0
                        )
                    dst = st[dkey]
                    t0 = th * 308
                    ps = ps1.tile([128, 308], F32, tag="p", name="p")
                    for kc in range(KC):
                        nc.tensor.matmul(
                            ps[:],
                            w_sb[wname][kc][:, ec * 128 : (ec + 1) * 128],
                            st["xt"][kc][:, t0 : t0 + 308],
                            start=(kc == 0),
                            stop=(kc == KC - 1),
                        )
                    nc.scalar.activation(
                        dst[:, ec, t0 : t0 + 308], ps[:], AF.Identity,
                        bias=bias[:, ec : ec + 1], scale=scale,
                    )
                return run

            def p_v(ti, half):
                def run():
                    toff, tw = TT[ti]
                    if ti not in st["vtok"]:
                        vt = vtp.tile([128, VW], ATTN_DT, tag="vtok", name="vtok")
                        st["vtok"][ti] = vt
                        # ones columns written here; the per-seq scatter DMAs
                        # copy full 780-wide rows so they carry the ones along
                        nc.gpsimd.memset(
                            vt[:].rearrange("t (h v) -> t h v", v=VST)[:, :, 64:65],
                            1.0,
                        )
                    vt = st["vtok"][ti]
                    pv = ps1.tile([128, 384], F32, tag="p", name="p")
                    for kc in range(KC):
                        nc.tensor.matmul(
                            pv[0:tw, :],
                            st["xt"][kc][:, toff : toff + tw],
                            w_sb["wv"][kc][:, half * 384 : (half + 1) * 384],
                            start=(kc == 0),
                            stop=(kc == KC - 1),
                        )
                    # partition-aligned PSUM->SBUF move (engines cannot start
                    # at an unaligned partition, so no per-seq split here)
                    nc.scalar.copy(
                        vt[0:tw].rearrange("t (h v) -> t h v", v=VST)[
                            :, half * 6 : half * 6 + 6, 0:64
                        ],
                        pv[0:tw].rearrange("t (h v) -> t h v", v=64),
                    )
                return run

            def p_vscatter(ti):
                def run():
                    toff, tw = TT[ti]
                    vt = st["vtok"].pop(ti)
                    # DMAs have no partition-alignment rules. SWDGE (gpsimd)
                    # descriptor generation has ring flow control (blocks when
                    # full), unlike the HWDGE dynamic path whose ring wrap
                    # clobbers in-flight descriptors.
                    for sq, r0, rs, n in _seq_pieces(toff, tw):
                        nc.gpsimd.dma_start(
                            st["vaug"][sq][rs : rs + n, :], vt[r0 : r0 + n, :]
                        )
                return run

            if c == 0:
                # startup: weights arrive in wq, wk, wv order over ~25us of
                # serialized DMA -- consume them in that order
                pieces = [p_transpose(ti) for ti in range(len(TT))]
                for th in range(2):
                    for ec in range(KC):
                        pieces.append(p_proj("wq", "q_sb", bq_col, SCALE, ec, th))
                for th in range(2):
                    for ec in range(KC):
                        pieces.append(p_proj("wk", "k_sb", bk_col, 1.0, ec, th))
                for ti in range(len(TT)):
                    pieces.append(p_v(ti, 0))
                    pieces.append(p_v(ti, 1))
                    pieces.append(p_vscatter(ti))
                return st, pieces, []
            # steady state: each v tile directly behind its own x-transpose
            # tile so the transpose psum ping-pong always has matmul work
            # between drains; q/k th=0 only needs transpose tiles 0-2
            pieces = [
                p_transpose(0), p_transpose(1),
                p_v(0, 0), p_transpose(2), p_v(0, 1), p_vscatter(0),
                p_v(1, 0), p_transpose(3), p_v(1, 1), p_vscatter(1),
                p_v(2, 0), p_transpose(4), p_v(2, 1), p_vscatter(2),
            ]
            pieces.extend([p_v(3, 0), p_v(3, 1), p_vscatter(3)])
            pieces.extend([p_v(4, 0), p_v(4, 1), p_vscatter(4)])
            for ec in range(KC):
                pieces.append(p_proj("wq", "q_sb", bq_col, SCALE, ec, 0))
                pieces.append(p_proj("wk", "k_sb", bk_col, 1.0, ec, 0))
            for ec in range(KC):
                pieces.append(p_proj("wq", "q_sb", bq_col, SCALE, ec, 1))
                pieces.append(p_proj("wk", "k_sb", bk_col, 1.0, ec, 1))
            return st, pieces, []

        def emit_attn(c, st):
            ast = {}
            ao_t = aop.tile([128, KC, CHUNK_TOK], ATTN_DT, tag="ao", name="ao")
            ast["ao"] = ao_t
            bstate = {}

            def p_scores(bb):
                def run():
                    q_sb, k_sb = st["q_sb"], st["k_sb"]
                    boff = bb * S
                    # one 2-bank psum tile; parity p at column p*512 so every
                    # per-head matmul output stays inside a single bank
                    sps = pss.tile([S, 1024], F32, tag="s", name="s")
                    for kc in range(KC):
                        for par in range(2):
                            nc.tensor.matmul(
                                sps[:, par * 512 + kc * SST : par * 512 + kc * SST + SST],
                                k_sb[par * 64 : par * 64 + 64, kc, boff : boff + S],
                                q_sb[par * 64 : par * 64 + 64, kc, boff : boff + SST],
                                start=True,
                                stop=True,
                            )
                    # unmasked exp straight from PSUM (one wide ACT op), then
                    # one in-place all-fp16 mask multiply on DVE (fast path)
                    sc = scp.tile([S, 2 * HHALF], ATTN_DT, tag="sc", name="sc")
                    nc.scalar.activation(
                        sc[:].rearrange("t (p c) -> t p c", p=2),
                        sps[:].rearrange("t (p c) -> t p c", p=2)[:, :, 0:HHALF],
                        AF.Exp,
                    )
                    nc.vector.tensor_mul(sc[:], sc[:], mask16_flat[:])
                    bstate[bb] = sc
                return run

            def p_out(bb):
                def run():
                    sc = bstate.pop(bb)
                    vaug = st["vaug"][bb]
                    boff = bb * S
                    aps = [
                        psa.tile([VST, HHALF], F32, tag="a", name="a")
                        for _ in range(2)
                    ]
                    for kc in range(KC):
                        for par in range(2):
                            h = 2 * kc + par
                            nc.tensor.matmul(
                                aps[par][:, kc * SST : kc * SST + SST],
                                vaug[0:S, h * VST : (h + 1) * VST],
                                sc[:, par * HHALF + kc * SST : par * HHALF + kc * SST + SST],
                                start=True,
                                stop=True,
                            )
                    recip = rcp.tile([1, 2 * HHALF], ATTN_DT, tag="recip", name="recip")
                    recbc = rcp.tile([64, 2 * HHALF], ATTN_DT, tag="recbc", name="recbc")
                    with nc.allow_low_precision(reason="softmax denom rounding"):
                        for par in range(2):
                            nc.vector.reciprocal(
                                recip[:, par * HHALF : (par + 1) * HHALF],
                                aps[par][64:65, :],
                            )
                    for par in range(2):
                        nc.gpsimd.partition_broadcast(
                            recbc[:, par * HHALF : (par + 1) * HHALF],
                            recip[:, par * HHALF : (par + 1) * HHALF],
                        )
                    for par in range(2):
                        nc.vector.tensor_mul(
                            ao_t[par * 64 : par * 64 + 64, :, boff : boff + S],
                            aps[par][0:64, :].rearrange(
                                "p (j s) -> p j s", s=SST
                            )[:, :, 0:S],
                            recbc[:, par * HHALF : (par + 1) * HHALF].rearrange(
                                "p (j s) -> p j s", s=SST
                            )[:, :, 0:S],
                        )
                return run

            s_pieces = [p_scores(bb) for bb in range(CHUNK_B)]
            o_pieces = [p_out(bb) for bb in range(CHUNK_B)]
            return ast, s_pieces, o_pieces

        def emit_final(c, ast):
            ctok = c * CHUNK_TOK
            ao = ast["ao"]

            def p_tt(ti):
                def run():
                    toff, tw = TT[ti]
                    yt = yp.tile([128, E], F32, tag="y", name="y")
                    for half in range(2):
                        yps = ps1.tile([128, 384], F32, tag="p", name="p")
                        for kc in range(KC):
                            nc.tensor.matmul(
                                yps[0:tw, :],
                                ao[:, kc, toff : toff + tw],
                                wo16[kc][:, half * 384 : (half + 1) * 384],
                                start=(kc == 0),
                                stop=(kc == KC - 1),
                            )
                        nc.vector.tensor_add(
                            yt[0:tw, half * 384 : (half + 1) * 384],
                            yps[0:tw, :],
                            bo_bc[0:tw, half * 384 : (half + 1) * 384],
                        )
                    # store on SP: emitted right after the y-add, so its
                    # data-ready wait is short and cannot starve the queue
                    # (keeping it off Pool protects the bcast/memset stream)
                    nc.sync.dma_start(
                        out[ctok + toff : ctok + toff + tw, :], yt[0:tw, :]
                    )
                return run

            return [p_tt(ti) for ti in range(len(TT))]

        # ---- interleave: attention batches of chunk c-1 are the spine;
        # projections of chunk c fill between; final tiles of chunk c-1 are
        # inserted as soon as the attention batches they read have been
        # emitted (one batch of slack for the normalize chain); the last two
        # final tiles carry over to the next step's head as tail fill ----
        qkv_st = {}
        held = []
        for c in range(NCHUNK + 1):
            fill = list(held)
            held = []
            spine = []
            after_o = {}
            if c < NCHUNK:
                qkv_st[c], p, ptail = emit_load_qkv(c)
                fill.extend(p)
                fill.extend(ptail)
            if 1 <= c <= NCHUNK:
                ast, s_p, o_p = emit_attn(c - 1, qkv_st.pop(c - 1))
                f = emit_final(c - 1, ast)
                # final tile ti reads ao of seqs up to (toff+tw-1)//S; insert
                # one O later for the DVE normalize to land; hold the last two
                ins_at = []
                for ti, (toff, tw) in enumerate(TT):
                    ins_at.append(min((toff + tw - 1) // S + 1, CHUNK_B - 1))
                for ti in range(2):
                    after_o.setdefault(ins_at[ti], []).append(f[ti])
                held.extend(f[2:])
                # spine: scores(bb+1) emitted before out(bb)
                spine = [("S", 0, s_p[0])]
                for bb in range(1, CHUNK_B):
                    spine.append(("S", bb, s_p[bb]))
                    spine.append(("O", bb - 1, o_p[bb - 1]))
                spine.append(("O", CHUNK_B - 1, o_p[CHUNK_B - 1]))

            if not spine:
                for p_ in fill:
                    p_()
            else:
                # x DMA + first transpose early
                nhead = min(1, len(fill))
                for p_ in fill[:nhead]:
                    p_()
                rest = fill[nhead:]
                k = 0
                for i, (kind, bb, sp) in enumerate(spine):
                    sp()
                    if kind == "O":
                        for fp_ in after_o.get(bb, ()):
                            fp_()
                    share = ((i + 1) * len(rest)) // len(spine)
                    while k < share:
                        rest[k]()
                        k += 1
                while k < len(rest):
                    rest[k]()
                    k += 1
            if c == 0:
                emit_bias_fold()
        for p_ in held:
            p_()

    nc.finalize()
    return nc


_NC_CACHE = {}


def get_nc():
    if "nc" not in _NC_CACHE:
        _NC_CACHE["nc"] = build_nc()
    return _NC_CACHE["nc"]


def kernel(**inputs):
    x = np.asarray(inputs["x"], dtype=np.float32)  # [512, 77, 768]
    nc = get_nc()
    shared = {
        k: np.asarray(inputs[k], dtype=np.float32)
        for k in ("wq", "bq", "wk", "bk", "wv", "bv", "wo", "bo")
    }
    in_maps = []
    for c in range(NCORES):
        m = dict(shared)
        m["x"] = np.ascontiguousarray(
            x[c * B_LOC : (c + 1) * B_LOC].reshape(NTOK, E)
        )
        in_maps.append(m)
    res = run_bass_kernel_spmd(nc, in_maps, core_ids=list(range(NCORES)))
    out = np.concatenate(
        [r_["out"].reshape(B_LOC, S, E) for r_ in res.results], axis=0
    )
    return out


# revision 76
# speedup vs baseline: 1.0230x; 1.0012x over previous
"""Causal multi-head attention block (B=512, S=77, H=12, D=64, E=768) on 8 trn2 cores.

Data parallel over batch: 64 sequences per core, weights replicated.
Cost-model timeline: ~447 us per core (baseline 566 us); hw rel err 3.8e-4.

Per-core dataflow (chunks of 8 sequences = 616 tokens, 8 chunks):
  - x loaded token-major f32r, transposed on PE with an f32r identity
    (1.5 cyc/row vs 2.0 for f32) to feature-major x^T
  - q^T, k^T = W^T @ x^T feature-major in f32r (1 cyc/row at moving >= 256);
    scale+bias folded into the ACT PSUM->SBUF copies (q carries the 1/8
    attention scale); fp16 outputs with one zero pad column so the scores
    matmuls can stream SST=78 columns (initializes the psum pad column)
  - v computed token-major in 128-token tiles (stationary = x^T slices,
    moving = wv halves; 23k cyc/chunk vs 37k for per-seq tiles), copied
    PSUM->SBUF into per-tile staging with 65-column head stride: column 64
    of each head block is a ones column, so the attention-out matmul's 65th
    output row is the softmax denominator for free
  - staging is scattered into per-seq v tiles [77, 12*65] by gpsimd/SWDGE
    DMAs: engines cannot access partitions at unaligned starts, HWDGE
    dynamic DMAs drop descriptors when their scratch ring wraps under load,
    but the SWDGE path has ring flow control and is race-free
  - scoresT[t,s] per (seq, head); heads packed by parity into separate PSUM
    banks of one 2-bank tile (parity at column p*512; base-partition-0 and
    base-partition-64 matmuls race row-groups on a shared bank port)
  - unmasked exp straight from PSUM on ACT (scores are bounded, no
    max-subtraction), then one in-place all-fp16 multiplicative causal mask
    on DVE (2-byte fast path) -- the PE->DVE->ACT->PE chain of the additive
    mask becomes PE->ACT->DVE->PE with ~half the latency
  - out matmuls consume the UNNORMALIZED masked exp with the augmented v as
    stationary -> [65, 78] per head: rows 0-63 = attn-out^T, row 64 = denom
  - reciprocal on DVE (fp16), broadcast across the 64 d-partitions on the
    otherwise-idle Pool engine (partition_broadcast), normalization
    multiplied in at the DVE PSUM->SBUF move of the attention output -> the
    PE never touches denominators or broadcasts
  - v bias folded through attention (softmax rows sum to 1):
    y = (attn @ v) @ wo + (bv @ wo + bo), bv@wo computed on-device
  - final projection in fp16 (ao fp16 stationary, wo fp16 moving, loaded
    via casting DMAs so no ACT convert blocks on the late wo load); bo
    added by DVE; y stores issued on SP right after their y-add so their
    data-ready wait never starves a DMA queue (all of Pool stays free for
    the scatter/broadcast stream)

Scheduling: per-chunk attention batches are the spine (scores of batch b+1
emitted before out of batch b); projection work of chunk c+1 fills between
them, x-transpose tiles interleaved with v tiles to keep the transpose psum
ping-pong fed; final tiles of chunk c-1 are inserted as soon as the
attention batches they read have been emitted, with the last tiles carried
to the next step's head as dependency-free tail fill.
"""

import sys

sys.path.insert(0, "/opt/trn_rl_repo")

import numpy as np
from contextlib import ExitStack

import concourse.bass as bass
import concourse.tile as tile
from concourse import bacc, mybir
from concourse.bass_utils import run_bass_kernel_spmd
from concourse.masks import make_identity

B, S, H, D = 512, 77, 12, 64
E = H * D  # 768
NCORES = 8
B_LOC = B // NCORES  # 64
NTOK = B_LOC * S  # 4928
CHUNK_B = 8
CHUNK_TOK = CHUNK_B * S  # 616
NCHUNK = B_LOC // CHUNK_B  # 8
KC = E // 128  # 6 k-chunks of 128
F32 = mybir.dt.float32
F32R = mybir.dt.float32r
FP16 = mybir.dt.float16
ATTN_DT = FP16
SCALE = 0.125
MASK_VAL = -1e9
SST = 78  # head block stride in scores layout (8B-aligned psum columns)
HHALF = 6 * SST  # 468, six heads per psum bank
VST = 65  # head stride in v_aug (64 v columns + ones column)
VW = H * VST  # 780

# token tiles within a chunk
TT = [(0, 128), (128, 128), (256, 128), (384, 128), (512, CHUNK_TOK - 512)]

ALU = mybir.AluOpType
AF = mybir.ActivationFunctionType


def bcast_ap(handle_ap, parts, n):
    """DRAM [n] vector viewed as [parts, n] with partition step 0."""
    return bass.AP(
        tensor=handle_ap.tensor,
        offset=handle_ap.offset,
        ap=[[0, parts]] + list(handle_ap.ap),
    )


def _seq_pieces(toff, tw):
    """Split chunk-token rows [toff, toff+tw) at sequence boundaries.
    Returns (seq_idx, row0_in_tile, row0_in_seq, nrows) pieces."""
    out = []
    r = toff
    while r < toff + tw:
        sq = r // S
        rs = r % S
        n = min(S - rs, toff + tw - r)
        out.append((sq, r - toff, rs, n))
        r += n
    return out


def build_nc():
    nc = bacc.Bacc("TRN2", target_bir_lowering=False)
    x = nc.dram_tensor("x", [NTOK, E], F32R, kind="ExternalInput").ap()
    wq = nc.dram_tensor("wq", [E, E], F32R, kind="ExternalInput").ap()
    wk = nc.dram_tensor("wk", [E, E], F32R, kind="ExternalInput").ap()
    wv = nc.dram_tensor("wv", [E, E], F32R, kind="ExternalInput").ap()
    wo = nc.dram_tensor("wo", [E, E], F32R, kind="ExternalInput").ap()
    bq = nc.dram_tensor("bq", [E], F32, kind="ExternalInput").ap()
    bk = nc.dram_tensor("bk", [E], F32, kind="ExternalInput").ap()
    bv = nc.dram_tensor("bv", [E], F32, kind="ExternalInput").ap()
    bo = nc.dram_tensor("bo", [E], F32, kind="ExternalInput").ap()
    out = nc.dram_tensor("out", [NTOK, E], F32, kind="ExternalOutput").ap()

    with tile.TileContext(nc) as tc, ExitStack() as ctx:
        singles = ctx.enter_context(tc.tile_pool(name="singles", bufs=1))
        xtokp = ctx.enter_context(tc.tile_pool(name="xtok", bufs=4))
        xtp = ctx.enter_context(tc.tile_pool(name="xt", bufs=2))
        qkp = ctx.enter_context(tc.tile_pool(name="qk", bufs=4))
        vp = ctx.enter_context(tc.tile_pool(name="v", bufs=16))
        vtp = ctx.enter_context(tc.tile_pool(name="vtok", bufs=3))
        aop = ctx.enter_context(tc.tile_pool(name="ao", bufs=2))
        scp = ctx.enter_context(tc.tile_pool(name="sc", bufs=2))
        rcp = ctx.enter_context(tc.tile_pool(name="rc", bufs=2))
        yp = ctx.enter_context(tc.tile_pool(name="y", bufs=2))
        ps1 = ctx.enter_context(tc.tile_pool(name="ps1", bufs=4, space="PSUM"))
        pss = ctx.enter_context(tc.tile_pool(name="pss", bufs=1, space="PSUM"))
        psa = ctx.enter_context(tc.tile_pool(name="psa", bufs=2, space="PSUM"))

        # ---- identity first: the first transposes must not wait on the
        # weight/bias DMA preamble ----
        ident = singles.tile([128, 128], F32, tag="ident", name="ident")
        make_identity(nc, ident[:])
        identr = singles.tile([128, 128], F32R, tag="identr", name="identr")
        nc.vector.tensor_copy(identr[:], ident[:])

        # ---- prefetch chunk 0 x tiles before the (large) weight DMAs ----
        xtok0 = []
        for toff, tw in TT:
            xtok = xtokp.tile([128, E], F32R, tag="xtok", name="xtok")
            nc.sync.dma_start(xtok[0:tw, :], x[toff : toff + tw, :])
            xtok0.append(xtok)

        # ---- weights ----
        w_sb = {}
        for name, w in (("wq", wq), ("wk", wk), ("wv", wv)):
            tiles = []
            for kc in range(KC):
                t = singles.tile([128, E], F32R, tag=f"{name}{kc}", name=f"{name}{kc}")
                nc.sync.dma_start(t[:], w[kc * 128 : (kc + 1) * 128, :])
                tiles.append(t)
            w_sb[name] = tiles
        # wo straight to fp16 via casting DMAs on the SWDGE path (keeps the
        # ACT queue free of converts that would head-of-line block on the
        # late wo DMA)
        wo16 = []
        for kc in range(KC):
            t16 = singles.tile([128, E], FP16, tag=f"wo16_{kc}", name=f"wo16_{kc}")
            nc.gpsimd.dma_start(t16[:], wo[kc * 128 : (kc + 1) * 128, :])
            wo16.append(t16)

        bq_col = singles.tile([128, KC], F32, tag="bqc", name="bqc")
        bk_col = singles.tile([128, KC], F32, tag="bkc", name="bkc")
        nc.gpsimd.dma_start(bq_col[:], bq.rearrange("(f p) -> p f", p=128))
        nc.gpsimd.dma_start(bk_col[:], bk.rearrange("(f p) -> p f", p=128))
        # fold the attention scale into the q bias: q = (x@wq)*s + bq*s
        nc.vector.tensor_scalar_mul(bq_col[:], bq_col[:], SCALE)

        bv_col = singles.tile([128, KC], FP16, tag="bvc", name="bvc")
        nc.gpsimd.dma_start(bv_col[:], bv.rearrange("(f p) -> p f", p=128))
        bo_bc = singles.tile([128, E], F32, tag="bob", name="bob")
        nc.gpsimd.dma_start(bo_bc[:], bcast_ap(bo, 128, E))

        # multiplicative causal mask in [t, (h s)] layout, fp16:
        # 1 where t <= s else 0 (applied AFTER the exp; scores are bounded so
        # the unmasked exp cannot overflow)
        mask16 = singles.tile([S, H, SST], FP16, tag="mask16", name="mask16")
        nc.gpsimd.memset(mask16[:], 1.0)
        nc.gpsimd.affine_select(
            out=mask16[:],
            in_=mask16[:],
            compare_op=ALU.is_ge,
            fill=0.0,
            base=0,
            pattern=[[0, H], [1, SST]],
            channel_multiplier=-1,
        )
        mask16_flat = mask16[:].rearrange("t h s -> t (h s)")

        # softmax rows sum to 1, so the v bias passes through attention:
        # y = (attn @ v) @ wo + (bv @ wo + bo).  Fold bv@wo into the bias.
        bvwo_row = singles.tile([1, E], F32, tag="bvwo", name="bvwo")

        def emit_bias_fold():
            for half in range(2):
                pbw = ps1.tile([1, 384], F32, tag="p", name="p")
                for kc in range(KC):
                    nc.tensor.matmul(
                        pbw[:], bv_col[:, kc : kc + 1],
                        wo16[kc][:, half * 384 : (half + 1) * 384],
                        start=(kc == 0), stop=(kc == KC - 1),
                    )
                nc.vector.tensor_copy(
                    bvwo_row[:, half * 384 : (half + 1) * 384], pbw[:]
                )
            bvwo_bc = yp.tile([128, E], F32, tag="y", name="bvwo_bc")
            nc.gpsimd.partition_broadcast(bvwo_bc[:], bvwo_row[:])
            nc.vector.tensor_add(bo_bc[:], bo_bc[:], bvwo_bc[:])

        # ---- per-chunk pieces ----
        def emit_load_qkv(c):
            ctok = c * CHUNK_TOK
            st = {}
            pieces = []

            xt_t = xtp.tile([128, KC, CHUNK_TOK], F32R, tag="xt", name="xt")
            st["xt"] = [xt_t[:, kc, :] for kc in range(KC)]
            st["vaug"] = [
                vp.tile([S, VW], ATTN_DT, tag="v", name="v") for _ in range(CHUNK_B)
            ]
            st["vtok"] = {}

            def p_transpose(ti):
                def run():
                    toff, tw = TT[ti]
                    if c == 0:
                        xtok = xtok0[ti]
                    else:
                        xtok = xtokp.tile([128, E], F32R, tag="xtok", name="xtok")
                        nc.sync.dma_start(
                            xtok[0:tw, :], x[ctok + toff : ctok + toff + tw, :]
                        )
                    for g in range(3):
                        tp = ps1.tile([128, 256], F32R, tag="p", name="tp")
                        for j in range(2):
                            kc = 2 * g + j
                            nc.tensor.transpose(
                                tp[:, j * 128 : j * 128 + tw],
                                xtok[0:tw, kc * 128 : (kc + 1) * 128],
                                identr[0:tw, 0:tw],
                            )
                        # small psum groups + alternating drain engines keep
                        # the transpose ping-pong fed
                        src = tp[:].rearrange("p (j c) -> p j c", c=128)[:, :, 0:tw]
                        dst = xt_t[:, 2 * g : 2 * g + 2, toff : toff + tw]
                        if g % 2 == 0:
                            nc.vector.tensor_copy(dst, src)
                        else:
                            nc.scalar.copy(dst, src)
                return run

            def p_proj(wname, dkey, bias, scale, ec, th):
                def run():
                    if dkey not in st:
                        st[dkey] = qkp.tile(
                            [128, KC, CHUNK_TOK + 1], ATTN_DT, tag="qk", name=dkey
                        )
                        # the scores matmuls stream 78 moving columns so the
                        # psum pad column is initialized; the last batch
                        # needs this extra zero column
                        nc.vector.memset(
                            st[dkey][:, :, CHUNK_TOK : CHUNK_TOK + 1], 0.0
                        )
                    dst = st[dkey]
                    t0 = th * 308
                    ps = ps1.tile([128, 308], F32, tag="p", name="p")
                    for kc in range(KC):
                        nc.tensor.matmul(
                            ps[:],
                            w_sb[wname][kc][:, ec * 128 : (ec + 1) * 128],
                            st["xt"][kc][:, t0 : t0 + 308],
                            start=(kc == 0),
                            stop=(kc == KC - 1),
                        )
                    nc.scalar.activation(
                        dst[:, ec, t0 : t0 + 308], ps[:], AF.Identity,
                        bias=bias[:, ec : ec + 1], scale=scale,
                    )
                return run

            def p_v(ti, half):
                def run():
                    toff, tw = TT[ti]
                    if ti not in st["vtok"]:
                        vt = vtp.tile([128, VW], ATTN_DT, tag="vtok", name="vtok")
                        st["vtok"][ti] = vt
                        # ones columns written here; the per-seq scatter DMAs
                        # copy full 780-wide rows so they carry the ones along
                        nc.gpsimd.memset(
                            vt[:].rearrange("t (h v) -> t h v", v=VST)[:, :, 64:65],
                            1.0,
                        )
                    vt = st["vtok"][ti]
                    pv = ps1.tile([128, 384], F32, tag="p", name="p")
                    for kc in range(KC):
                        nc.tensor.matmul(
                            pv[0:tw, :],
                            st["xt"][kc][:, toff : toff + tw],
                            w_sb["wv"][kc][:, half * 384 : (half + 1) * 384],
                            start=(kc == 0),
                            stop=(kc == KC - 1),
                        )
                    # partition-aligned PSUM->SBUF move (engines cannot start
                    # at an unaligned partition, so no per-seq split here)
                    nc.scalar.copy(
                        vt[0:tw].rearrange("t (h v) -> t h v", v=VST)[
                            :, half * 6 : half * 6 + 6, 0:64
                        ],
                        pv[0:tw].rearrange("t (h v) -> t h v", v=64),
                    )
                return run

            def p_vscatter(ti):
                def run():
                    toff, tw = TT[ti]
                    vt = st["vtok"].pop(ti)
                    # DMAs have no partition-alignment rules. SWDGE (gpsimd)
                    # descriptor generation has ring flow control (blocks when
                    # full), unlike the HWDGE dynamic path whose ring wrap
                    # clobbers in-flight descriptors.
                    for sq, r0, rs, n in _seq_pieces(toff, tw):
                        nc.gpsimd.dma_start(
                            st["vaug"][sq][rs : rs + n, :], vt[r0 : r0 + n, :]
                        )
                return run

            if c == 0:
                # startup: weights arrive in wq, wk, wv order over ~25us of
                # serialized DMA -- consume them in that order
                pieces = [p_transpose(ti) for ti in range(len(TT))]
                for th in range(2):
                    for ec in range(KC):
                        pieces.append(p_proj("wq", "q_sb", bq_col, SCALE, ec, th))
                for th in range(2):
                    for ec in range(KC):
                        pieces.append(p_proj("wk", "k_sb", bk_col, 1.0, ec, th))
                for ti in range(len(TT)):
                    pieces.append(p_v(ti, 0))
                    pieces.append(p_v(ti, 1))
                    pieces.append(p_vscatter(ti))
                return st, pieces, []
            # steady state: each v tile directly behind its own x-transpose
            # tile so the transpose psum ping-pong always has matmul work
            # between drains; q/k th=0 only needs transpose tiles 0-2
            pieces = [
                p_transpose(0), p_transpose(1),
                p_v(0, 0), p_transpose(2), p_v(0, 1), p_vscatter(0),
                p_v(1, 0), p_transpose(3), p_v(1, 1), p_vscatter(1),
                p_v(2, 0), p_transpose(4), p_v(2, 1), p_vscatter(2),
            ]
            pieces.extend([p_v(3, 0), p_v(3, 1), p_vscatter(3)])
            pieces.extend([p_v(4, 0), p_v(4, 1), p_vscatter(4)])
            for ec in range(KC):
                pieces.append(p_proj("wq", "q_sb", bq_col, SCALE, ec, 0))
                pieces.append(p_proj("wk", "k_sb", bk_col, 1.0, ec, 0))
            for ec in range(KC):
                pieces.append(p_proj("wq", "q_sb", bq_col, SCALE, ec, 1))
                pieces.append(p_proj("wk", "k_sb", bk_col, 1.0, ec, 1))
            return st, pieces, []

        def emit_attn(c, st):
            ast = {}
            ao_t = aop.tile([128, KC, CHUNK_TOK], ATTN_DT, tag="ao", name="ao")
            ast["ao"] = ao_t
            bstate = {}

            def p_scores(bb):
                def run():
                    q_sb, k_sb = st["q_sb"], st["k_sb"]
                    boff = bb * S
                    # one 2-bank psum tile; parity p at column p*512 so every
                    # per-head matmul output stays inside a single bank
                    sps = pss.tile([S, 1024], F32, tag="s", name="s")
                    for kc in range(KC):
                        for par in range(2):
                            nc.tensor.matmul(
                                sps[:, par * 512 + kc * SST : par * 512 + kc * SST + SST],
                                k_sb[par * 64 : par * 64 + 64, kc, boff : boff + S],
                                q_sb[par * 64 : par * 64 + 64, kc, boff : boff + SST],
                                start=True,
                                stop=True,
                            )
                    # unmasked exp straight from PSUM (one wide ACT op), then
                    # one in-place all-fp16 mask multiply on DVE (fast path)
                    sc = scp.tile([S, 2 * HHALF], ATTN_DT, tag="sc", name="sc")
                    nc.scalar.activation(
                        sc[:].rearrange("t (p c) -> t p c", p=2),
                        sps[:].rearrange("t (p c) -> t p c", p=2)[:, :, 0:HHALF],
                        AF.Exp,
                    )
                    nc.vector.tensor_mul(sc[:], sc[:], mask16_flat[:])
                    bstate[bb] = sc
                return run

            def p_out(bb):
                def run():
                    sc = bstate.pop(bb)
                    vaug = st["vaug"][bb]
                    boff = bb * S
                    aps = [
                        psa.tile([VST, HHALF], F32, tag="a", name="a")
                        for _ in range(2)
                    ]
                    for kc in range(KC):
                        for par in range(2):
                            h = 2 * kc + par
                            nc.tensor.matmul(
                                aps[par][:, kc * SST : kc * SST + SST],
                                vaug[0:S, h * VST : (h + 1) * VST],
                                sc[:, par * HHALF + kc * SST : par * HHALF + kc * SST + SST],
                                start=True,
                                stop=True,
                            )
                    recip = rcp.tile([1, 2 * HHALF], ATTN_DT, tag="recip", name="recip")
                    recbc = rcp.tile([64, 2 * HHALF], ATTN_DT, tag="recbc", name="recbc")
                    with nc.allow_low_precision(reason="softmax denom rounding"):
                        for par in range(2):
                            nc.vector.reciprocal(
                                recip[:, par * HHALF : (par + 1) * HHALF],
                                aps[par][64:65, :],
                            )
                    for par in range(2):
                        nc.gpsimd.partition_broadcast(
                            recbc[:, par * HHALF : (par + 1) * HHALF],
                            recip[:, par * HHALF : (par + 1) * HHALF],
                        )
                    for par in range(2):
                        nc.vector.tensor_mul(
                            ao_t[par * 64 : par * 64 + 64, :, boff : boff + S],
                            aps[par][0:64, :].rearrange(
                                "p (j s) -> p j s", s=SST
                            )[:, :, 0:S],
                            recbc[:, par * HHALF : (par + 1) * HHALF].rearrange(
                                "p (j s) -> p j s", s=SST
                            )[:, :, 0:S],
                        )
                return run

            s_pieces = [p_scores(bb) for bb in range(CHUNK_B)]
            o_pieces = [p_out(bb) for bb in range(CHUNK_B)]
            return ast, s_pieces, o_pieces

        def emit_final(c, ast):
            ctok = c * CHUNK_TOK
            ao = ast["ao"]

            def p_tt(ti):
                def run():
                    toff, tw = TT[ti]
                    yt = yp.tile([128, E], F32, tag="y", name="y")
                    for half in range(2):
                        yps = ps1.tile([128, 384], F32, tag="p", name="p")
                        for kc in range(KC):
                            nc.tensor.matmul(
                                yps[0:tw, :],
                                ao[:, kc, toff : toff + tw],
                                wo16[kc][:, half * 384 : (half + 1) * 384],
                                start=(kc == 0),
                                stop=(kc == KC - 1),
                            )
                        nc.vector.tensor_add(
                            yt[0:tw, half * 384 : (half + 1) * 384],
                            yps[0:tw, :],
                            bo_bc[0:tw, half * 384 : (half + 1) * 384],
                        )
                    # store on SP: emitted right after the y-add, so its
                    # data-ready wait is short and cannot starve the queue
                    # (keeping it off Pool protects the bcast/memset stream)
                    nc.sync.dma_start(
                        out[ctok + toff : ctok + toff + tw, :], yt[0:tw, :]
                    )
                return run

            return [p_tt(ti) for ti in range(len(TT))]

        # ---- interleave: attention batches of chunk c-1 are the spine;
        # projections of chunk c fill between; final tiles of chunk c-1 are
        # inserted as soon as the attention batches they read have been
        # emitted (one batch of slack for the normalize chain); the last two
        # final tiles carry over to the next step's head as tail fill ----
        qkv_st = {}
        held = []
        for c in range(NCHUNK + 1):
            fill = list(held)
            held = []
            spine = []
            after_o = {}
            if c < NCHUNK:
                qkv_st[c], p, ptail = emit_load_qkv(c)
                fill.extend(p)
                fill.extend(ptail)
            if 1 <= c <= NCHUNK:
                ast, s_p, o_p = emit_attn(c - 1, qkv_st.pop(c - 1))
                f = emit_final(c - 1, ast)
                # final tile ti reads ao of seqs up to (toff+tw-1)//S; insert
                # one O later for the DVE normalize to land; hold the last two
                ins_at = []
                for ti, (toff, tw) in enumerate(TT):
                    ins_at.append(min((toff + tw - 1) // S + 1, CHUNK_B - 1))
                for ti in range(2):
                    after_o.setdefault(ins_at[ti], []).append(f[ti])
                held.extend(f[2:])
                # spine: scores(bb+1) emitted before out(bb)
                spine = [("S", 0, s_p[0])]
                for bb in range(1, CHUNK_B):
                    spine.append(("S", bb, s_p[bb]))
                    spine.append(("O", bb - 1, o_p[bb - 1]))
                spine.append(("O", CHUNK_B - 1, o_p[CHUNK_B - 1]))

            if not spine:
                for p_ in fill:
                    p_()
            else:
                # x DMA + first transpose early
                nhead = min(1, len(fill))
                for p_ in fill[:nhead]:
                    p_()
                rest = fill[nhead:]
                k = 0
                for i, (kind, bb, sp) in enumerate(spine):
                    if kind == "O" and k < len(rest):
                        # one fill piece directly ahead of each out batch so
                        # the PE has work in hand while the softmax chain of
                        # this batch drains
                        rest[k]()
                        k += 1
                    sp()
                    if kind == "O":
                        for fp_ in after_o.get(bb, ()):
                            fp_()
                    share = ((i + 1) * len(rest)) // len(spine)
                    while k < share:
                        rest[k]()
                        k += 1
                while k < len(rest):
                    rest[k]()
                    k += 1
            if c == 0:
                emit_bias_fold()
        for p_ in held:
            p_()

    nc.finalize()
    return nc


_NC_CACHE = {}


def get_nc():
    if "nc" not in _NC_CACHE:
        _NC_CACHE["nc"] = build_nc()
    return _NC_CACHE["nc"]


def kernel(**inputs):
    x = np.asarray(inputs["x"], dtype=np.float32)  # [512, 77, 768]
    nc = get_nc()
    shared = {
        k: np.asarray(inputs[k], dtype=np.float32)
        for k in ("wq", "bq", "wk", "bk", "wv", "bv", "wo", "bo")
    }
    in_maps = []
    for c in range(NCORES):
        m = dict(shared)
        m["x"] = np.ascontiguousarray(
            x[c * B_LOC : (c + 1) * B_LOC].reshape(NTOK, E)
        )
        in_maps.append(m)
    res = run_bass_kernel_spmd(nc, in_maps, core_ids=list(range(NCORES)))
    out = np.concatenate(
        [r_["out"].reshape(B_LOC, S, E) for r_ in res.results], axis=0
    )
    return out
